# revision 10
# baseline (speedup 1.0000x reference)
"""Adaptive embedding (4-bucket) lookup + projection on 8 TRN2 NeuronCores.

Strategy: the device program is a pure streaming GEMM; all index work is host
side.  Host gathers token rows, uploads dense d-major eT matrices in bf16
plus pre-transposed sqrt(D)-scaled projections.  Work split:
  t0 (d=1024): proj0 dout-halves x token-quarters (2-way model parallel)
  t1 (d=256):  same 2-way split
  t2 (d=64):   token-parallel, halves on partitions 0-63 / 64-127
  t3 (d=16):   token-parallel, halves at partitions 0-15 / 32-47
Core: out[dout_block, tok] = projT_block.T @ eT accumulated in PSUM;
DVE/ACT alternate evacuating PSUM; t2/t3 outputs cast to fp8e4 in the evac
(~31% of output norm -> ~1.5% global rel err vs the 2e-2 budget), halving
the dominant store traffic.  Filler matmuls pad PE-stream gaps so the HAM
clock gate reaches and keeps 2.4 GHz.  Host transposes dout-major results
back to token order.
"""

import os
import sys

import numpy as np

for _p in ("/opt/trn_rl_repo",):
    if _p not in sys.path:
        sys.path.insert(0, _p)

import ml_dtypes

BF16 = ml_dtypes.bfloat16
FP8 = ml_dtypes.float8_e4m3

N_TOKEN = 267735
CUTS = (0, 20000, 40000, 200000, N_TOKEN)
D_OUT = 1024
EMB_SCALE = float(D_OUT) ** 0.5
N_CORES = 8
P = 128

_PROGRAM_CACHE = {}
LAST_RESULTS = None  # BassKernelResults of the most recent run (for profiling)


def _chunks(n, m=512):
    out = []
    c = 0
    while c < n:
        out.append((c, min(c + m, n)))
        c += m
    return out


def _build_program(n0q, n1q, n2h, n3h):
    import concourse.bacc as bacc
    import concourse.mybir as mybir
    import concourse.tile as tile

    dt = mybir.dt
    nc = bacc.Bacc("TRN2", target_bir_lowering=False, debug=False)

    n2c, n3c = 2 * n2h, 2 * n3h
    assert n0q <= 512 and n1q <= 512

    warm = nc.dram_tensor("warm", [P, P], dt.bfloat16, kind="ExternalInput")
    in3 = nc.dram_tensor("in3", [48, 1024 + n3h], dt.bfloat16,
                         kind="ExternalInput")
    in2 = nc.dram_tensor("in2", [P, 1024 + n2h], dt.bfloat16,
                         kind="ExternalInput")
    # in0 halves: [p0 k0-3 | e0 k0-3] then [p0 k4-7 | e0 k4-7]
    x0h = 2048 + 4 * n0q
    in0 = nc.dram_tensor("in0", [P, 2 * x0h], dt.bfloat16,
                         kind="ExternalInput")
    in1 = nc.dram_tensor("in1", [P, 1024 + 2 * n1q], dt.bfloat16,
                         kind="ExternalInput")

    o0 = nc.dram_tensor("o0", [P, 4, n0q], dt.bfloat16, kind="ExternalOutput")
    o1 = nc.dram_tensor("o1", [P, 4, n1q], dt.bfloat16, kind="ExternalOutput")
    o2 = nc.dram_tensor("o2", [P, 8, n2c], dt.float8e4, kind="ExternalOutput")
    o3 = nc.dram_tensor("o3", [P, 8, n3c], dt.float8e4, kind="ExternalOutput")

    with tile.TileContext(nc) as tc:
        with (
            tc.tile_pool(name="io", bufs=1) as io,
            tc.tile_pool(name="psum", bufs=7, space="PSUM") as pp,
            tc.tile_pool(name="psw", bufs=1, space="PSUM") as ppw,
        ):
            # --- loads, in PE-consumption order, all on the SP ring
            warm_sb = io.tile([P, P], dt.bfloat16, tag="warm")
            nc.sync.dma_start(warm_sb[:], warm[:])
            in3_sb = io.tile([48, 1024 + n3h], dt.bfloat16, tag="in3")
            nc.sync.dma_start(in3_sb[:], in3[:])
            in2_sb = io.tile([P, 1024 + n2h], dt.bfloat16, tag="in2")
            nc.sync.dma_start(in2_sb[:], in2[:])
            in0_sb = io.tile([P, 2 * x0h], dt.bfloat16, tag="in0")
            nc.sync.dma_start(in0_sb[:, 0:x0h], in0[:, 0:x0h])
            in1_sb = io.tile([P, 1024 + 2 * n1q], dt.bfloat16, tag="in1")
            nc.sync.dma_start(in1_sb[:], in1[:])
            nc.sync.dma_start(in0_sb[:, x0h:], in0[:, x0h:])

            # --- output staging (dout-major)
            st0 = io.tile([P, 4, n0q], dt.bfloat16, tag="st0")
            st1 = io.tile([P, 4, n1q], dt.bfloat16, tag="st1")
            st2 = io.tile([P, 8, n2c], dt.float8e4, tag="st2")
            st3 = io.tile([P, 8, n3c], dt.float8e4, tag="st3")

            flip = [0]

            def evac(dst, ps):
                # DVE is a bit faster than ACT on copies: 4 of every 7
                if flip[0] % 7 < 4:
                    nc.vector.tensor_copy(dst, ps)
                else:
                    nc.scalar.copy(dst, ps)
                flip[0] += 1

            def psum(name):
                return pp.tile([P, 512], mybir.dt.float32, tag="ps",
                               name=name)

            # --- PE fillers: junk matmuls into a dedicated PSUM bank (no
            # consumers, same-engine WAW only).  A run at the start warms
            # the HAM clock gate (2.4 GHz after ~3.4us of sustained
            # activity); singles pad evac-backpressure gaps in the t3/t2
            # stream so the activity monitor never sees the PE idle.
            psw = ppw.tile([P, 512], mybir.dt.float32, tag="w", name="w")

            def filler(n, cols=P):
                for _ in range(n):
                    nc.tensor.matmul(psw[:, 0:cols], warm_sb[:],
                                     warm_sb[:, 0:cols], start=True,
                                     stop=True)

            filler(12)

            # --- t3: d=16, row-tiles at partitions 0-15 / 32-47
            for s in range(8):
                for base, off in ((0, 0), (32, n3h)):
                    for c0, c1 in _chunks(n3h):
                        ps = psum(f"ps3_{s}_{base}_{c0}")
                        nc.tensor.matmul(
                            ps[:, 0:c1 - c0],
                            in3_sb[base:base + 16, s * P:(s + 1) * P],
                            in3_sb[base:base + 16, 1024 + c0:1024 + c1],
                            start=True, stop=True)
                        evac(st3[:, s, off + c0:off + c1], ps[:, 0:c1 - c0])
                filler(1)
            nc.sync.dma_start(o3[:], st3[:])

            # --- t2: d=64, row-tiles at partitions 0-63 / 64-127
            for s in range(8):
                for base, off in ((0, 0), (64, n2h)):
                    for c0, c1 in _chunks(n2h):
                        ps = psum(f"ps2_{s}_{base}_{c0}")
                        nc.tensor.matmul(
                            ps[:, 0:c1 - c0],
                            in2_sb[base:base + 64, s * P:(s + 1) * P],
                            in2_sb[base:base + 64, 1024 + c0:1024 + c1],
                            start=True, stop=True)
                        evac(st2[:, s, off + c0:off + c1], ps[:, 0:c1 - c0])
                    filler(1)
                if s == 3:
                    nc.sync.dma_start(o2[:, 0:4, :], st2[:, 0:4, :])
            nc.sync.dma_start(o2[:, 4:8, :], st2[:, 4:8, :])

            # --- t0 phase A: k-tiles 0-3 accumulate (in0 half A); banks
            # stay live through t1; phase B finishes k 4-7
            def p0_ap(k, s):
                base = (k // 4) * x0h
                return in0_sb[:, base + ((k % 4) * 4 + s) * P:
                              base + ((k % 4) * 4 + s + 1) * P]

            def e0_ap(k, c0, c1):
                base = (k // 4) * x0h + 2048
                return in0_sb[:, base + (k % 4) * n0q + c0:
                              base + (k % 4) * n0q + c1]

            ps0 = {}
            for s in range(4):
                for c0, c1 in _chunks(n0q):
                    ps = psum(f"ps0_{s}_{c0}")
                    ps0[(s, c0)] = ps
                    for k in range(4):
                        nc.tensor.matmul(ps[:, 0:c1 - c0], p0_ap(k, s),
                                         e0_ap(k, c0, c1),
                                         start=(k == 0), stop=False)

            # --- t1: d=256, 2 k-tiles, dout-half shard
            for s in range(4):
                for c0, c1 in _chunks(n1q):
                    ps = psum(f"ps1_{s}_{c0}")
                    for k in range(2):
                        nc.tensor.matmul(
                            ps[:, 0:c1 - c0],
                            in1_sb[:, (k * 4 + s) * P:(k * 4 + s + 1) * P],
                            in1_sb[:, 1024 + k * n1q + c0:
                                   1024 + k * n1q + c1],
                            start=(k == 0), stop=(k == 1))
                    evac(st1[:, s, c0:c1], ps[:, 0:c1 - c0])
            nc.sync.dma_start(o1[:], st1[:])

            # --- t0 phase B
            for s in range(4):
                for c0, c1 in _chunks(n0q):
                    ps = ps0[(s, c0)]
                    for k in range(4, 8):
                        nc.tensor.matmul(ps[:, 0:c1 - c0], p0_ap(k, s),
                                         e0_ap(k, c0, c1),
                                         start=False, stop=(k == 7))
                    evac(st0[:, s, c0:c1], ps[:, 0:c1 - c0])
            nc.sync.dma_start(o0[:], st0[:])

    nc.finalize()
    return nc


def _pad_cols(a, n):
    if a.shape[1] == n:
        return a
    out = np.zeros((a.shape[0], n), a.dtype)
    out[:, :a.shape[1]] = a
    return out


def kernel(inp, emb0, emb1, emb2, emb3, proj0, proj1, proj2, proj3):
    global LAST_RESULTS
    from concourse.bass_utils import run_bass_kernel_spmd

    flat = np.asarray(inp).reshape(-1).astype(np.int64)
    T = flat.shape[0]
    cuts = np.asarray(CUTS)
    tblid = np.searchsorted(cuts[1:], flat, side="right")
    embs = [np.asarray(e, np.float32) for e in (emb0, emb1, emb2, emb3)]
    projTs = [
        np.ascontiguousarray((np.asarray(p, np.float32) * EMB_SCALE).T)
        for p in (proj0, proj1, proj2, proj3)
    ]

    pos = {}
    loc = {}
    for t in range(4):
        pos[t] = np.nonzero(tblid == t)[0]
        loc[t] = flat[pos[t]] - cuts[t]

    n0q = max(1, -(-len(pos[0]) // 4))
    n1q = max(1, -(-len(pos[1]) // 4))
    n2h = max(1, -(-len(pos[2]) // 16))
    n3h = max(1, -(-len(pos[3]) // 16))
    n2c, n3c = 2 * n2h, 2 * n3h

    key = (n0q, n1q, n2h, n3h)
    nc = _PROGRAM_CACHE.get(key)
    if nc is None:
        nc = _build_program(*key)
        _PROGRAM_CACHE[key] = nc

    warm_np = np.zeros((P, P), BF16)

    e0_q = []
    for q in range(4):
        et = embs[0][loc[0][q::4]].T  # [1024, n]
        et = _pad_cols(et, n0q).reshape(8, P, n0q)
        e0_q.append(np.ascontiguousarray(et.transpose(1, 0, 2)).astype(BF16))
    pk0 = projTs[0].reshape(8, P, 8, P)  # [k, d_part, s_glob, c]
    p0_h = [
        np.ascontiguousarray(
            pk0[:, :, h * 4:(h + 1) * 4, :].transpose(1, 0, 2, 3)
        ).astype(BF16)
        for h in range(2)
    ]

    e1_q = []
    for q in range(4):
        et = embs[1][loc[1][q::4]].T  # [256, n]
        et = _pad_cols(et, n1q).reshape(2, P, n1q)
        e1_q.append(np.ascontiguousarray(et.transpose(1, 0, 2)).astype(BF16))
    pk1 = projTs[1].reshape(2, P, 8, P)
    p1_h = [
        np.ascontiguousarray(
            pk1[:, :, h * 4:(h + 1) * 4, :].transpose(1, 0, 2, 3)
        ).astype(BF16)
        for h in range(2)
    ]

    pk2 = projTs[2].reshape(64, 8 * P)
    p2 = np.concatenate([pk2, pk2], axis=0).astype(BF16)  # [128, 1024]
    pk3 = projTs[3].reshape(16, 8 * P)
    p3 = np.zeros((48, 8 * P), np.float32)
    p3[0:16] = pk3
    p3[32:48] = pk3
    p3 = p3.astype(BF16)

    in_maps = []
    core_meta = []
    for k in range(N_CORES):
        q, h = k // 2, k % 2

        p0 = p0_h[h]
        e0 = e0_q[q]
        in0 = np.concatenate([
            p0[:, 0:4].reshape(P, -1), e0[:, 0:4].reshape(P, -1),
            p0[:, 4:8].reshape(P, -1), e0[:, 4:8].reshape(P, -1),
        ], axis=1)

        in1 = np.concatenate([
            p1_h[h].reshape(P, -1), e1_q[q].reshape(P, -1)
        ], axis=1)

        rows2 = loc[2][k::8]
        nA2 = min(len(rows2), n2h)
        eA = _pad_cols(embs[2][rows2[:nA2]].T, n2h)
        eB = _pad_cols(embs[2][rows2[nA2:]].T, n2h)
        in2 = np.concatenate(
            [p2, np.concatenate([eA, eB], axis=0).astype(BF16)], axis=1)

        rows3 = loc[3][k::8]
        nA3 = min(len(rows3), n3h)
        e3 = np.zeros((48, n3h), np.float32)
        e3[0:16, :nA3] = embs[3][rows3[:nA3]].T
        e3[32:48, :len(rows3) - nA3] = embs[3][rows3[nA3:]].T
        in3 = np.concatenate([p3, e3.astype(BF16)], axis=1)

        in_maps.append({
            "warm": warm_np, "in0": np.ascontiguousarray(in0),
            "in1": np.ascontiguousarray(in1),
            "in2": np.ascontiguousarray(in2),
            "in3": np.ascontiguousarray(in3),
        })
        core_meta.append((nA2, nA3))

    trace = bool(os.environ.get("KERNEL_TRACE"))
    res = run_bass_kernel_spmd(nc, in_maps, core_ids=list(range(N_CORES)),
                               trace=trace)
    LAST_RESULTS = res

    out = np.empty((T, D_OUT), np.float32)

    for k in range(N_CORES):
        q, h = k // 2, k % 2
        r = res.results[k]
        for t, name in ((0, "o0"), (1, "o1")):
            positions = pos[t][q::4]
            n = len(positions)
            if n:
                dat = np.asarray(r[name])[:, :, :n].astype(np.float32)
                out[np.ix_(positions,
                           np.arange(h * 512, h * 512 + 512))] = (
                    dat.transpose(2, 1, 0).reshape(n, 512))
        nA2, nA3 = core_meta[k]
        p2k = pos[2][k::8]
        ob = np.asarray(r["o2"]).astype(np.float32)  # [128, 8, n2c]
        if nA2:
            out[p2k[:nA2]] = ob[:, :, :nA2].transpose(2, 1, 0).reshape(
                nA2, D_OUT)
        nB = len(p2k) - nA2
        if nB > 0:
            out[p2k[nA2:]] = ob[:, :, n2h:n2h + nB].transpose(2, 1, 0).reshape(
                nB, D_OUT)
        p3k = pos[3][k::8]
        ob = np.asarray(r["o3"]).astype(np.float32)  # [128, 8, n3c]
        if nA3:
            out[p3k[:nA3]] = ob[:, :, :nA3].transpose(2, 1, 0).reshape(
                nA3, D_OUT)
        nB = len(p3k) - nA3
        if nB > 0:
            out[p3k[nA3:]] = ob[:, :, n3h:n3h + nB].transpose(2, 1, 0).reshape(
                nB, D_OUT)

    return out.reshape(*np.asarray(inp).shape, D_OUT)


# revision 17
# speedup vs baseline: 1.1002x; 1.1002x over previous
"""Adaptive embedding (4-bucket) lookup + projection on 8 TRN2 NeuronCores.

Strategy: the device program is a pure streaming GEMM; all index work is host
side.  Host gathers token rows, uploads dense d-major eT matrices in bf16
plus pre-transposed sqrt(D)-scaled projections.  Work split:
  t0 (d=1024): proj0 dout-halves x token-quarters (2-way model parallel)
  t1 (d=256):  same 2-way split
  t2 (d=64):   token-parallel, halves on partitions 0-63 / 64-127
  t3 (d=16):   token-parallel, halves at partitions 0-15 / 32-47
Core: out[dout_block, tok] = projT_block.T @ eT accumulated in PSUM;
DVE/ACT alternate evacuating PSUM; t2/t3 outputs cast to fp8e4 in the evac
(~31% of output norm -> ~1.5% global rel err vs the 2e-2 budget), halving
the dominant store traffic.  Filler matmuls pad PE-stream gaps so the HAM
clock gate reaches and keeps 2.4 GHz.  Host transposes dout-major results
back to token order.
"""

import os
import sys

import numpy as np

for _p in ("/opt/trn_rl_repo",):
    if _p not in sys.path:
        sys.path.insert(0, _p)

import ml_dtypes

BF16 = ml_dtypes.bfloat16
FP8 = ml_dtypes.float8_e4m3

N_TOKEN = 267735
CUTS = (0, 20000, 40000, 200000, N_TOKEN)
D_OUT = 1024
EMB_SCALE = float(D_OUT) ** 0.5
N_CORES = 8
P = 128

_PROGRAM_CACHE = {}
LAST_RESULTS = None  # BassKernelResults of the most recent run (for profiling)


def _chunks(n, m=512):
    out = []
    c = 0
    while c < n:
        out.append((c, min(c + m, n)))
        c += m
    return out


def _build_program(n0q, n1q, n2h, n3h):
    import concourse.bacc as bacc
    import concourse.mybir as mybir
    import concourse.tile as tile

    dt = mybir.dt
    nc = bacc.Bacc("TRN2", target_bir_lowering=False, debug=False)

    n2c, n3c = 2 * n2h, 2 * n3h
    assert n0q <= 512 and n1q <= 512

    warm = nc.dram_tensor("warm", [P, P], dt.bfloat16, kind="ExternalInput")
    in3 = nc.dram_tensor("in3", [48, 1024 + n3h], dt.bfloat16,
                         kind="ExternalInput")
    in2 = nc.dram_tensor("in2", [P, 1024 + n2h], dt.bfloat16,
                         kind="ExternalInput")
    # in0 halves: [p0 k0-3 | e0 k0-3] then [p0 k4-7 | e0 k4-7]
    x0h = 2048 + 4 * n0q
    in0 = nc.dram_tensor("in0", [P, 2 * x0h], dt.bfloat16,
                         kind="ExternalInput")
    in1 = nc.dram_tensor("in1", [P, 1024 + 2 * n1q], dt.bfloat16,
                         kind="ExternalInput")

    o0 = nc.dram_tensor("o0", [P, 4, n0q], dt.bfloat16, kind="ExternalOutput")
    o1 = nc.dram_tensor("o1", [P, 4, n1q], dt.bfloat16, kind="ExternalOutput")
    o2 = nc.dram_tensor("o2", [P, 8, n2c], dt.bfloat16, kind="ExternalOutput")
    o3 = nc.dram_tensor("o3", [P, 8, n3c], dt.bfloat16, kind="ExternalOutput")

    with tile.TileContext(nc) as tc:
        with (
            tc.tile_pool(name="io", bufs=1) as io,
            tc.tile_pool(name="psv", bufs=4, space="PSUM") as pv,
            tc.tile_pool(name="psa", bufs=3, space="PSUM") as pa,
            tc.tile_pool(name="psw", bufs=1, space="PSUM") as ppw,
        ):
            # --- loads, in PE-consumption order, all on the SP ring
            warm_sb = io.tile([P, P], dt.bfloat16, tag="warm")
            nc.sync.dma_start(warm_sb[:], warm[:])
            in3_sb = io.tile([48, 1024 + n3h], dt.bfloat16, tag="in3")
            nc.sync.dma_start(in3_sb[:], in3[:])
            in2_sb = io.tile([P, 1024 + n2h], dt.bfloat16, tag="in2")
            nc.sync.dma_start(in2_sb[:], in2[:])
            in0_sb = io.tile([P, 2 * x0h], dt.bfloat16, tag="in0")
            nc.sync.dma_start(in0_sb[:, 0:x0h], in0[:, 0:x0h])
            in1_sb = io.tile([P, 1024 + 2 * n1q], dt.bfloat16, tag="in1")
            nc.sync.dma_start(in1_sb[:], in1[:])
            nc.sync.dma_start(in0_sb[:, x0h:], in0[:, x0h:])

            # --- output staging (dout-major)
            st0 = io.tile([P, 4, n0q], dt.bfloat16, tag="st0")
            st1 = io.tile([P, 4, n1q], dt.bfloat16, tag="st1")
            st2 = io.tile([P, 8, n2c], dt.bfloat16, tag="st2")
            st3 = io.tile([P, 8, n3c], dt.bfloat16, tag="st3")

            # PSUM quanta are routed to per-engine pools (DVE gets 4 of 7,
            # it is a bit faster than ACT) so the two evac chains never
            # share rotation slots -> no cross-engine WAR coupling.
            flip = [0]
            last_eng = [0]

            def psum(name):
                if flip[0] % 7 < 4:
                    ps = pv.tile([P, 512], mybir.dt.float32, tag="pv",
                                 name=name)
                    last_eng[0] = 0
                else:
                    ps = pa.tile([P, 512], mybir.dt.float32, tag="pa",
                                 name=name)
                    last_eng[0] = 1
                flip[0] += 1
                return ps, last_eng[0]

            def evac(dst, ps, eng):
                if eng == 0:
                    nc.vector.tensor_copy(dst, ps)
                else:
                    nc.scalar.copy(dst, ps)

            # --- PE fillers: junk matmuls into a dedicated PSUM bank (no
            # consumers, same-engine WAW only).  A run at the start warms
            # the HAM clock gate (2.4 GHz after ~3.4us of sustained
            # activity); singles pad evac-backpressure gaps in the t3/t2
            # stream so the activity monitor never sees the PE idle.
            psw = ppw.tile([P, 512], mybir.dt.float32, tag="w", name="w")

            def filler(n, cols=P):
                for _ in range(n):
                    nc.tensor.matmul(psw[:, 0:cols], warm_sb[:],
                                     warm_sb[:, 0:cols], start=True,
                                     stop=True)

            filler(12)

            # --- t3: d=16, row-tiles at partitions 0-15 / 32-47
            for s in range(8):
                for base, off in ((0, 0), (32, n3h)):
                    for c0, c1 in _chunks(n3h):
                        ps, eng = psum(f"ps3_{s}_{base}_{c0}")
                        nc.tensor.matmul(
                            ps[:, 0:c1 - c0],
                            in3_sb[base:base + 16, s * P:(s + 1) * P],
                            in3_sb[base:base + 16, 1024 + c0:1024 + c1],
                            start=True, stop=True)
                        evac(st3[:, s, off + c0:off + c1], ps[:, 0:c1 - c0],
                             eng)
            nc.sync.dma_start(o3[:], st3[:])

            # --- t2: d=64, row-tiles at partitions 0-63 / 64-127
            for s in range(8):
                for base, off in ((0, 0), (64, n2h)):
                    for c0, c1 in _chunks(n2h):
                        ps, eng = psum(f"ps2_{s}_{base}_{c0}")
                        nc.tensor.matmul(
                            ps[:, 0:c1 - c0],
                            in2_sb[base:base + 64, s * P:(s + 1) * P],
                            in2_sb[base:base + 64, 1024 + c0:1024 + c1],
                            start=True, stop=True)
                        evac(st2[:, s, off + c0:off + c1], ps[:, 0:c1 - c0],
                             eng)
                if s == 3:
                    nc.sync.dma_start(o2[:, 0:4, :], st2[:, 0:4, :])
            nc.sync.dma_start(o2[:, 4:8, :], st2[:, 4:8, :])

            # --- t0 phase A: k-tiles 0-3 accumulate (in0 half A); banks
            # stay live through t1; phase B finishes k 4-7
            def p0_ap(k, s):
                base = (k // 4) * x0h
                return in0_sb[:, base + ((k % 4) * 4 + s) * P:
                              base + ((k % 4) * 4 + s + 1) * P]

            def e0_ap(k, c0, c1):
                base = (k // 4) * x0h + 2048
                return in0_sb[:, base + (k % 4) * n0q + c0:
                              base + (k % 4) * n0q + c1]

            ps0 = {}
            for s in range(4):
                for c0, c1 in _chunks(n0q):
                    ps, eng = psum(f"ps0_{s}_{c0}")
                    ps0[(s, c0)] = (ps, eng)
                    for k in range(4):
                        nc.tensor.matmul(ps[:, 0:c1 - c0], p0_ap(k, s),
                                         e0_ap(k, c0, c1),
                                         start=(k == 0), stop=False)

            # --- t1: d=256, 2 k-tiles, dout-half shard
            for s in range(4):
                for c0, c1 in _chunks(n1q):
                    ps, eng = psum(f"ps1_{s}_{c0}")
                    for k in range(2):
                        nc.tensor.matmul(
                            ps[:, 0:c1 - c0],
                            in1_sb[:, (k * 4 + s) * P:(k * 4 + s + 1) * P],
                            in1_sb[:, 1024 + k * n1q + c0:
                                   1024 + k * n1q + c1],
                            start=(k == 0), stop=(k == 1))
                    evac(st1[:, s, c0:c1], ps[:, 0:c1 - c0], eng)
            nc.sync.dma_start(o1[:], st1[:])

            # --- t0 phase B
            for s in range(4):
                for c0, c1 in _chunks(n0q):
                    ps, eng = ps0[(s, c0)]
                    for k in range(4, 8):
                        nc.tensor.matmul(ps[:, 0:c1 - c0], p0_ap(k, s),
                                         e0_ap(k, c0, c1),
                                         start=False, stop=(k == 7))
                    evac(st0[:, s, c0:c1], ps[:, 0:c1 - c0], eng)
                if s == 1:
                    nc.sync.dma_start(o0[:, 0:2, :], st0[:, 0:2, :])
            nc.sync.dma_start(o0[:, 2:4, :], st0[:, 2:4, :])

    nc.finalize()
    return nc


def _pad_cols(a, n):
    if a.shape[1] == n:
        return a
    out = np.zeros((a.shape[0], n), a.dtype)
    out[:, :a.shape[1]] = a
    return out


def kernel(inp, emb0, emb1, emb2, emb3, proj0, proj1, proj2, proj3):
    global LAST_RESULTS
    from concourse.bass_utils import run_bass_kernel_spmd

    flat = np.asarray(inp).reshape(-1).astype(np.int64)
    T = flat.shape[0]
    cuts = np.asarray(CUTS)
    tblid = np.searchsorted(cuts[1:], flat, side="right")
    embs = [np.asarray(e, np.float32) for e in (emb0, emb1, emb2, emb3)]
    projTs = [
        np.ascontiguousarray((np.asarray(p, np.float32) * EMB_SCALE).T)
        for p in (proj0, proj1, proj2, proj3)
    ]

    pos = {}
    loc = {}
    for t in range(4):
        pos[t] = np.nonzero(tblid == t)[0]
        loc[t] = flat[pos[t]] - cuts[t]

    n0q = max(1, -(-len(pos[0]) // 4))
    n1q = max(1, -(-len(pos[1]) // 4))
    n2h = max(1, -(-len(pos[2]) // 16))
    n3h = max(1, -(-len(pos[3]) // 16))
    n2c, n3c = 2 * n2h, 2 * n3h

    key = (n0q, n1q, n2h, n3h)
    nc = _PROGRAM_CACHE.get(key)
    if nc is None:
        nc = _build_program(*key)
        _PROGRAM_CACHE[key] = nc

    warm_np = np.zeros((P, P), BF16)

    e0_q = []
    for q in range(4):
        et = embs[0][loc[0][q::4]].T  # [1024, n]
        et = _pad_cols(et, n0q).reshape(8, P, n0q)
        e0_q.append(np.ascontiguousarray(et.transpose(1, 0, 2)).astype(BF16))
    pk0 = projTs[0].reshape(8, P, 8, P)  # [k, d_part, s_glob, c]
    p0_h = [
        np.ascontiguousarray(
            pk0[:, :, h * 4:(h + 1) * 4, :].transpose(1, 0, 2, 3)
        ).astype(BF16)
        for h in range(2)
    ]

    e1_q = []
    for q in range(4):
        et = embs[1][loc[1][q::4]].T  # [256, n]
        et = _pad_cols(et, n1q).reshape(2, P, n1q)
        e1_q.append(np.ascontiguousarray(et.transpose(1, 0, 2)).astype(BF16))
    pk1 = projTs[1].reshape(2, P, 8, P)
    p1_h = [
        np.ascontiguousarray(
            pk1[:, :, h * 4:(h + 1) * 4, :].transpose(1, 0, 2, 3)
        ).astype(BF16)
        for h in range(2)
    ]

    pk2 = projTs[2].reshape(64, 8 * P)
    p2 = np.concatenate([pk2, pk2], axis=0).astype(BF16)  # [128, 1024]
    pk3 = projTs[3].reshape(16, 8 * P)
    p3 = np.zeros((48, 8 * P), np.float32)
    p3[0:16] = pk3
    p3[32:48] = pk3
    p3 = p3.astype(BF16)

    in_maps = []
    core_meta = []
    for k in range(N_CORES):
        q, h = k // 2, k % 2

        p0 = p0_h[h]
        e0 = e0_q[q]
        in0 = np.concatenate([
            p0[:, 0:4].reshape(P, -1), e0[:, 0:4].reshape(P, -1),
            p0[:, 4:8].reshape(P, -1), e0[:, 4:8].reshape(P, -1),
        ], axis=1)

        in1 = np.concatenate([
            p1_h[h].reshape(P, -1), e1_q[q].reshape(P, -1)
        ], axis=1)

        rows2 = loc[2][k::8]
        nA2 = min(len(rows2), n2h)
        eA = _pad_cols(embs[2][rows2[:nA2]].T, n2h)
        eB = _pad_cols(embs[2][rows2[nA2:]].T, n2h)
        in2 = np.concatenate(
            [p2, np.concatenate([eA, eB], axis=0).astype(BF16)], axis=1)

        rows3 = loc[3][k::8]
        nA3 = min(len(rows3), n3h)
        e3 = np.zeros((48, n3h), np.float32)
        e3[0:16, :nA3] = embs[3][rows3[:nA3]].T
        e3[32:48, :len(rows3) - nA3] = embs[3][rows3[nA3:]].T
        in3 = np.concatenate([p3, e3.astype(BF16)], axis=1)

        in_maps.append({
            "warm": warm_np, "in0": np.ascontiguousarray(in0),
            "in1": np.ascontiguousarray(in1),
            "in2": np.ascontiguousarray(in2),
            "in3": np.ascontiguousarray(in3),
        })
        core_meta.append((nA2, nA3))

    trace = bool(os.environ.get("KERNEL_TRACE"))
    res = run_bass_kernel_spmd(nc, in_maps, core_ids=list(range(N_CORES)),
                               trace=trace)
    LAST_RESULTS = res

    out = np.empty((T, D_OUT), np.float32)

    for k in range(N_CORES):
        q, h = k // 2, k % 2
        r = res.results[k]
        for t, name in ((0, "o0"), (1, "o1")):
            positions = pos[t][q::4]
            n = len(positions)
            if n:
                dat = np.asarray(r[name])[:, :, :n].astype(np.float32)
                out[np.ix_(positions,
                           np.arange(h * 512, h * 512 + 512))] = (
                    dat.transpose(2, 1, 0).reshape(n, 512))
        nA2, nA3 = core_meta[k]
        p2k = pos[2][k::8]
        ob = np.asarray(r["o2"]).astype(np.float32)  # [128, 8, n2c]
        if nA2:
            out[p2k[:nA2]] = ob[:, :, :nA2].transpose(2, 1, 0).reshape(
                nA2, D_OUT)
        nB = len(p2k) - nA2
        if nB > 0:
            out[p2k[nA2:]] = ob[:, :, n2h:n2h + nB].transpose(2, 1, 0).reshape(
                nB, D_OUT)
        p3k = pos[3][k::8]
        ob = np.asarray(r["o3"]).astype(np.float32)  # [128, 8, n3c]
        if nA3:
            out[p3k[:nA3]] = ob[:, :, :nA3].transpose(2, 1, 0).reshape(
                nA3, D_OUT)
        nB = len(p3k) - nA3
        if nB > 0:
            out[p3k[nA3:]] = ob[:, :, n3h:n3h + nB].transpose(2, 1, 0).reshape(
                nB, D_OUT)

    return out.reshape(*np.asarray(inp).shape, D_OUT)


# revision 18
# speedup vs baseline: 1.1587x; 1.0531x over previous
"""Adaptive embedding (4-bucket) lookup + projection on 8 TRN2 NeuronCores.

Strategy: the device program is a pure streaming GEMM; all index work is host
side.  Host gathers token rows, uploads dense d-major eT matrices in bf16
plus pre-transposed sqrt(D)-scaled projections.  Work split:
  t0 (d=1024): proj0 dout-halves x token-quarters (2-way model parallel)
  t1 (d=256):  same 2-way split
  t2 (d=64):   token-parallel, halves on partitions 0-63 / 64-127
  t3 (d=16):   token-parallel, halves at partitions 0-15 / 32-47
Core: out[dout_block, tok] = projT_block.T @ eT accumulated in PSUM;
DVE/ACT alternate evacuating PSUM; t2/t3 outputs cast to fp8e4 in the evac
(~31% of output norm -> ~1.5% global rel err vs the 2e-2 budget), halving
the dominant store traffic.  Filler matmuls pad PE-stream gaps so the HAM
clock gate reaches and keeps 2.4 GHz.  Host transposes dout-major results
back to token order.
"""

import os
import sys

import numpy as np

for _p in ("/opt/trn_rl_repo",):
    if _p not in sys.path:
        sys.path.insert(0, _p)

import ml_dtypes

BF16 = ml_dtypes.bfloat16
FP8 = ml_dtypes.float8_e4m3

N_TOKEN = 267735
CUTS = (0, 20000, 40000, 200000, N_TOKEN)
D_OUT = 1024
EMB_SCALE = float(D_OUT) ** 0.5
N_CORES = 8
P = 128

_PROGRAM_CACHE = {}
LAST_RESULTS = None  # BassKernelResults of the most recent run (for profiling)


def _chunks(n, m=512):
    out = []
    c = 0
    while c < n:
        out.append((c, min(c + m, n)))
        c += m
    return out


def _build_program(n0q, n1q, n2h, n3h):
    import concourse.bacc as bacc
    import concourse.mybir as mybir
    import concourse.tile as tile

    dt = mybir.dt
    nc = bacc.Bacc("TRN2", target_bir_lowering=False, debug=False)

    n2c, n3c = 2 * n2h, 2 * n3h
    assert n0q <= 512 and n1q <= 512

    warm = nc.dram_tensor("warm", [P, P], dt.bfloat16, kind="ExternalInput")
    in3 = nc.dram_tensor("in3", [48, 1024 + n3h], dt.bfloat16,
                         kind="ExternalInput")
    in2 = nc.dram_tensor("in2", [P, 1024 + n2h], dt.bfloat16,
                         kind="ExternalInput")
    # in0 halves: [p0 k0-3 | e0 k0-3] then [p0 k4-7 | e0 k4-7]
    x0h = 2048 + 4 * n0q
    in0 = nc.dram_tensor("in0", [P, 2 * x0h], dt.bfloat16,
                         kind="ExternalInput")
    in1 = nc.dram_tensor("in1", [P, 1024 + 2 * n1q], dt.bfloat16,
                         kind="ExternalInput")

    o0 = nc.dram_tensor("o0", [P, 4, n0q], dt.bfloat16, kind="ExternalOutput")
    o1 = nc.dram_tensor("o1", [P, 4, n1q], dt.bfloat16, kind="ExternalOutput")
    o2 = nc.dram_tensor("o2", [P, 8, n2c], dt.bfloat16, kind="ExternalOutput")
    o3 = nc.dram_tensor("o3", [P, 8, n3c], dt.bfloat16, kind="ExternalOutput")

    with tile.TileContext(nc) as tc:
        with (
            tc.tile_pool(name="io", bufs=1) as io,
            tc.tile_pool(name="psum", bufs=8, space="PSUM") as pp,
        ):
            # --- loads, in PE-consumption order, all on the SP ring
            warm_sb = io.tile([P, P], dt.bfloat16, tag="warm")
            nc.sync.dma_start(warm_sb[:], warm[:])
            in3_sb = io.tile([48, 1024 + n3h], dt.bfloat16, tag="in3")
            nc.sync.dma_start(in3_sb[:], in3[:])
            in2_sb = io.tile([P, 1024 + n2h], dt.bfloat16, tag="in2")
            nc.sync.dma_start(in2_sb[:], in2[:])
            in0_sb = io.tile([P, 2 * x0h], dt.bfloat16, tag="in0")
            nc.sync.dma_start(in0_sb[:, 0:x0h], in0[:, 0:x0h])
            in1_sb = io.tile([P, 1024 + 2 * n1q], dt.bfloat16, tag="in1")
            nc.sync.dma_start(in1_sb[:], in1[:])
            nc.sync.dma_start(in0_sb[:, x0h:], in0[:, x0h:])

            # --- output staging (dout-major)
            st0 = io.tile([P, 4, n0q], dt.bfloat16, tag="st0")
            st1 = io.tile([P, 4, n1q], dt.bfloat16, tag="st1")
            st2 = io.tile([P, 8, n2c], dt.bfloat16, tag="st2")
            st3 = io.tile([P, 8, n3c], dt.bfloat16, tag="st3")

            flip = [0]

            def psum(name):
                ps = pp.tile([P, 512], mybir.dt.float32, tag="ps", name=name)
                eng = 0 if flip[0] % 7 < 4 else 1
                flip[0] += 1
                return ps, eng

            def evac(dst, ps, eng):
                if eng == 0:
                    nc.vector.tensor_copy(dst, ps)
                else:
                    nc.scalar.copy(dst, ps)

            # --- PE warm-up fillers (cover the initial load latency)
            for i in range(12):
                psw, _ = psum(f"w{i}")
                nc.tensor.matmul(psw[:, 0:P], warm_sb[:], warm_sb[:],
                                 start=True, stop=True)

            # --- t3: d=16, row-tiles at partitions 0-15 / 32-47
            for s in range(8):
                for base, off in ((0, 0), (32, n3h)):
                    for c0, c1 in _chunks(n3h):
                        ps, eng = psum(f"ps3_{s}_{base}_{c0}")
                        nc.tensor.matmul(
                            ps[:, 0:c1 - c0],
                            in3_sb[base:base + 16, s * P:(s + 1) * P],
                            in3_sb[base:base + 16, 1024 + c0:1024 + c1],
                            start=True, stop=True)
                        evac(st3[:, s, off + c0:off + c1], ps[:, 0:c1 - c0],
                             eng)
            nc.sync.dma_start(o3[:], st3[:])

            # --- t2: d=64, row-tiles at partitions 0-63 / 64-127
            for s in range(8):
                for base, off in ((0, 0), (64, n2h)):
                    for c0, c1 in _chunks(n2h):
                        ps, eng = psum(f"ps2_{s}_{base}_{c0}")
                        nc.tensor.matmul(
                            ps[:, 0:c1 - c0],
                            in2_sb[base:base + 64, s * P:(s + 1) * P],
                            in2_sb[base:base + 64, 1024 + c0:1024 + c1],
                            start=True, stop=True)
                        evac(st2[:, s, off + c0:off + c1], ps[:, 0:c1 - c0],
                             eng)
                if s == 3:
                    nc.sync.dma_start(o2[:, 0:4, :], st2[:, 0:4, :])
            nc.sync.dma_start(o2[:, 4:8, :], st2[:, 4:8, :])

            # --- t0 phase A: k-tiles 0-3 accumulate (in0 half A); banks
            # stay live through t1; phase B finishes k 4-7
            def p0_ap(k, s):
                base = (k // 4) * x0h
                return in0_sb[:, base + ((k % 4) * 4 + s) * P:
                              base + ((k % 4) * 4 + s + 1) * P]

            def e0_ap(k, c0, c1):
                base = (k // 4) * x0h + 2048
                return in0_sb[:, base + (k % 4) * n0q + c0:
                              base + (k % 4) * n0q + c1]

            ps0 = {}
            for s in range(4):
                for c0, c1 in _chunks(n0q):
                    ps, eng = psum(f"ps0_{s}_{c0}")
                    ps0[(s, c0)] = (ps, eng)
                    for k in range(4):
                        nc.tensor.matmul(ps[:, 0:c1 - c0], p0_ap(k, s),
                                         e0_ap(k, c0, c1),
                                         start=(k == 0), stop=False)

            # --- t1: d=256, 2 k-tiles, dout-half shard
            for s in range(4):
                for c0, c1 in _chunks(n1q):
                    ps, eng = psum(f"ps1_{s}_{c0}")
                    for k in range(2):
                        nc.tensor.matmul(
                            ps[:, 0:c1 - c0],
                            in1_sb[:, (k * 4 + s) * P:(k * 4 + s + 1) * P],
                            in1_sb[:, 1024 + k * n1q + c0:
                                   1024 + k * n1q + c1],
                            start=(k == 0), stop=(k == 1))
                    evac(st1[:, s, c0:c1], ps[:, 0:c1 - c0], eng)
            nc.sync.dma_start(o1[:], st1[:])

            # --- t0 phase B
            for s in range(4):
                for c0, c1 in _chunks(n0q):
                    ps, eng = ps0[(s, c0)]
                    for k in range(4, 8):
                        nc.tensor.matmul(ps[:, 0:c1 - c0], p0_ap(k, s),
                                         e0_ap(k, c0, c1),
                                         start=False, stop=(k == 7))
                    evac(st0[:, s, c0:c1], ps[:, 0:c1 - c0], eng)
            nc.sync.dma_start(o0[:], st0[:])

    nc.finalize()
    return nc


def _pad_cols(a, n):
    if a.shape[1] == n:
        return a
    out = np.zeros((a.shape[0], n), a.dtype)
    out[:, :a.shape[1]] = a
    return out


def kernel(inp, emb0, emb1, emb2, emb3, proj0, proj1, proj2, proj3):
    global LAST_RESULTS
    from concourse.bass_utils import run_bass_kernel_spmd

    flat = np.asarray(inp).reshape(-1).astype(np.int64)
    T = flat.shape[0]
    cuts = np.asarray(CUTS)
    tblid = np.searchsorted(cuts[1:], flat, side="right")
    embs = [np.asarray(e, np.float32) for e in (emb0, emb1, emb2, emb3)]
    projTs = [
        np.ascontiguousarray((np.asarray(p, np.float32) * EMB_SCALE).T)
        for p in (proj0, proj1, proj2, proj3)
    ]

    pos = {}
    loc = {}
    for t in range(4):
        pos[t] = np.nonzero(tblid == t)[0]
        loc[t] = flat[pos[t]] - cuts[t]

    n0q = max(1, -(-len(pos[0]) // 4))
    n1q = max(1, -(-len(pos[1]) // 4))
    n2h = max(1, -(-len(pos[2]) // 16))
    n3h = max(1, -(-len(pos[3]) // 16))
    n2c, n3c = 2 * n2h, 2 * n3h

    key = (n0q, n1q, n2h, n3h)
    nc = _PROGRAM_CACHE.get(key)
    if nc is None:
        nc = _build_program(*key)
        _PROGRAM_CACHE[key] = nc

    warm_np = np.zeros((P, P), BF16)

    e0_q = []
    for q in range(4):
        et = embs[0][loc[0][q::4]].T  # [1024, n]
        et = _pad_cols(et, n0q).reshape(8, P, n0q)
        e0_q.append(np.ascontiguousarray(et.transpose(1, 0, 2)).astype(BF16))
    pk0 = projTs[0].reshape(8, P, 8, P)  # [k, d_part, s_glob, c]
    p0_h = [
        np.ascontiguousarray(
            pk0[:, :, h * 4:(h + 1) * 4, :].transpose(1, 0, 2, 3)
        ).astype(BF16)
        for h in range(2)
    ]

    e1_q = []
    for q in range(4):
        et = embs[1][loc[1][q::4]].T  # [256, n]
        et = _pad_cols(et, n1q).reshape(2, P, n1q)
        e1_q.append(np.ascontiguousarray(et.transpose(1, 0, 2)).astype(BF16))
    pk1 = projTs[1].reshape(2, P, 8, P)
    p1_h = [
        np.ascontiguousarray(
            pk1[:, :, h * 4:(h + 1) * 4, :].transpose(1, 0, 2, 3)
        ).astype(BF16)
        for h in range(2)
    ]

    pk2 = projTs[2].reshape(64, 8 * P)
    p2 = np.concatenate([pk2, pk2], axis=0).astype(BF16)  # [128, 1024]
    pk3 = projTs[3].reshape(16, 8 * P)
    p3 = np.zeros((48, 8 * P), np.float32)
    p3[0:16] = pk3
    p3[32:48] = pk3
    p3 = p3.astype(BF16)

    in_maps = []
    core_meta = []
    for k in range(N_CORES):
        q, h = k // 2, k % 2

        p0 = p0_h[h]
        e0 = e0_q[q]
        in0 = np.concatenate([
            p0[:, 0:4].reshape(P, -1), e0[:, 0:4].reshape(P, -1),
            p0[:, 4:8].reshape(P, -1), e0[:, 4:8].reshape(P, -1),
        ], axis=1)

        in1 = np.concatenate([
            p1_h[h].reshape(P, -1), e1_q[q].reshape(P, -1)
        ], axis=1)

        rows2 = loc[2][k::8]
        nA2 = min(len(rows2), n2h)
        eA = _pad_cols(embs[2][rows2[:nA2]].T, n2h)
        eB = _pad_cols(embs[2][rows2[nA2:]].T, n2h)
        in2 = np.concatenate(
            [p2, np.concatenate([eA, eB], axis=0).astype(BF16)], axis=1)

        rows3 = loc[3][k::8]
        nA3 = min(len(rows3), n3h)
        e3 = np.zeros((48, n3h), np.float32)
        e3[0:16, :nA3] = embs[3][rows3[:nA3]].T
        e3[32:48, :len(rows3) - nA3] = embs[3][rows3[nA3:]].T
        in3 = np.concatenate([p3, e3.astype(BF16)], axis=1)

        in_maps.append({
            "warm": warm_np, "in0": np.ascontiguousarray(in0),
            "in1": np.ascontiguousarray(in1),
            "in2": np.ascontiguousarray(in2),
            "in3": np.ascontiguousarray(in3),
        })
        core_meta.append((nA2, nA3))

    trace = bool(os.environ.get("KERNEL_TRACE"))
    res = run_bass_kernel_spmd(nc, in_maps, core_ids=list(range(N_CORES)),
                               trace=trace)
    LAST_RESULTS = res

    out = np.empty((T, D_OUT), np.float32)

    for k in range(N_CORES):
        q, h = k // 2, k % 2
        r = res.results[k]
        for t, name in ((0, "o0"), (1, "o1")):
            positions = pos[t][q::4]
            n = len(positions)
            if n:
                dat = np.asarray(r[name])[:, :, :n].astype(np.float32)
                out[np.ix_(positions,
                           np.arange(h * 512, h * 512 + 512))] = (
                    dat.transpose(2, 1, 0).reshape(n, 512))
        nA2, nA3 = core_meta[k]
        p2k = pos[2][k::8]
        ob = np.asarray(r["o2"]).astype(np.float32)  # [128, 8, n2c]
        if nA2:
            out[p2k[:nA2]] = ob[:, :, :nA2].transpose(2, 1, 0).reshape(
                nA2, D_OUT)
        nB = len(p2k) - nA2
        if nB > 0:
            out[p2k[nA2:]] = ob[:, :, n2h:n2h + nB].transpose(2, 1, 0).reshape(
                nB, D_OUT)
        p3k = pos[3][k::8]
        ob = np.asarray(r["o3"]).astype(np.float32)  # [128, 8, n3c]
        if nA3:
            out[p3k[:nA3]] = ob[:, :, :nA3].transpose(2, 1, 0).reshape(
                nA3, D_OUT)
        nB = len(p3k) - nA3
        if nB > 0:
            out[p3k[nA3:]] = ob[:, :, n3h:n3h + nB].transpose(2, 1, 0).reshape(
                nB, D_OUT)

    return out.reshape(*np.asarray(inp).shape, D_OUT)
